# revision 8
# baseline (speedup 1.0000x reference)
"""Causal attention kernel for Trainium2 (Bass/Tile), 8-core SPMD.

Problem: B=2, H=16, S=2048, D=64 fp32 attention with a causal mask.
Sharding: batch*heads = 32 slices -> 4 heads per core across 8 cores.

Per-core algorithm (heads processed in pairs stacked on SBUF partitions
0-63 / 64-127):
  S^T = K @ Q^T blockwise: [kblock=128, qtile=256]; the two heads' QK^T
  matmuls are emitted interleaved so they run concurrently on PE row
  tiles T0/T8 ((64,128) mode - disjoint SBUF partitions, disjoint PSUM
  banks), halving QK^T wall time. Both heads' S^T for a 2-kblock group
  live in ONE [128, 1024] PSUM tile (A cols 0-511 | B cols 512-1023).
  P^T = exp(S^T/8) in bf16, split across two engines per group:
    ACT:  activation(Exp) -> bf16 (exact)
    DVE:  exp2 bit-trick: bf16 = bitcast(int16(S^T*a + b)), 3% max err.
  Masking is FUSED into the DVE op for diagonal (mixed) groups:
  scalar_tensor_tensor adds a per-position bias tile (b where allowed,
  -1e30 where masked); the f32->i16 convert saturates to 0x8000 = -0.0
  in bf16, contributing exactly 0 to PV and the denominator.
  out^T = V_aug^T @ P^T in bf16 accumulated in PSUM; V_aug has a ones
  column -> row 64 = softmax denominator. Host divides + transposes.
  PSUM->SBUF evacuation alternates ACT/DVE; DMA out per qtile.

Rationale: the previous version was ACT-engine-bound (~85us: 61us exp
+ 24us instruction overhead on one engine). v2 splits exp across
ACT+DVE (~40us each), halves QK^T via PE row tiling, and removes all
mask DMA/multiply traffic (fused bias trick). Tensor port floor for
this dataflow is ~46us.
"""

import sys

import numpy as np

for _p in ('/opt/trn_rl_repo', '/root/.axon_site/_ro/trn_rl_repo'):
    if _p not in sys.path:
        sys.path.append(_p)

B, H, S, D = 2, 16, 2048, 64
NCORES = 8
HPC = (B * H) // NCORES  # heads per core = 4
QT = 256                 # q tile
KB = 128                 # k block (partition dim)
GSIZE = 2                # kblocks per group
NQT = S // QT            # 8
NKB = S // KB            # 16

LOG2E = float(np.log2(np.e))
SCALE = 0.125            # 1/sqrt(D)
A16 = (2.0 ** 7) * LOG2E * SCALE
C16 = 0.0434609
B16 = (2.0 ** 7) * (127.0 - C16)
NEG = -1.0e30

# full groups routed to the DVE exp-trick for ACT/DVE load balance
import os as _os
_DVE_X = int(_os.environ.get('DVE_X', '0'))
DVE_FULL = {(j, 0) for j in range(2, NQT)} | {(j, 1) for j in range(5, NQT)}
if _DVE_X == 1:
    DVE_FULL |= {(j, 2) for j in range(6, NQT)}
elif _DVE_X == -1:
    DVE_FULL = {(j, 0) for j in range(2, NQT)}

_CACHE = {}
import os
_NO_PIPE = bool(int(os.environ.get('NO_PIPE', '0')))
_DBG = bool(int(os.environ.get('KDBG', '0')))


def _plan_from_mask(mask):
    """plan[j] = tuple of groups (i0, w, kind, pat); pat indexes tiles.

    tiles[pat] = ('causal', rel, w) for on-chip-generatable causal bias
    patterns, or ('data', arr[KB, 2*w*QT] f32) bias tiles (b / -1e30,
    duplicated for the two heads).
    """
    plan = []
    tiles = []
    tile_idx = {}
    for j in range(NQT):
        row = []
        mq = mask[j * QT:(j + 1) * QT]  # [QT, S]
        nkb_here = 0
        for i0 in range(0, NKB, GSIZE):
            w = min(GSIZE, NKB - i0)
            blk = [mq[:, i * KB:(i + 1) * KB].any() for i in range(i0, i0 + w)]
            if not any(blk):
                continue
            lead = blk.index(True)
            i0 += lead
            w = len(blk) - lead - blk[::-1].index(True)
            R = mq[:, i0 * KB:(i0 + w) * KB]  # [QT, w*KB]
            if R.all():
                row.append((i0, w, 'full', 0))
                continue
            # bias tile in S^T layout: t[x, gp*QT + y] = R[y, gp*KB + x]
            Mt = np.where(R.T, np.float32(B16), np.float32(NEG))
            Mt = Mt.reshape(w, KB, QT).transpose(1, 0, 2).reshape(KB, w * QT)
            key = Mt.tobytes()
            if key not in tile_idx:
                tile_idx[key] = len(tiles)
                qq = np.arange(j * QT, (j + 1) * QT)[None, :]
                kk = np.arange(i0 * KB, (i0 + w) * KB)[:, None]
                if np.array_equal(R.T, qq >= kk):
                    tiles.append(('causal', i0 * KB - j * QT, w))
                else:
                    tiles.append(
                        ('data', np.concatenate([Mt, Mt], axis=1)))
            row.append((i0, w, 'mixed', tile_idx[key]))
        plan.append(tuple(row))
    return tuple(plan), tiles


def _stack_bias_tiles(tiles):
    data = [t[1] for t in tiles if t[0] == 'data']
    if not data:
        return None
    out = np.full((len(data), KB, 2 * GSIZE * QT), np.float32(B16),
                  dtype=np.float32)
    for i, t in enumerate(data):
        out[i, :, :t.shape[1]] = t
    return out


def _build(plan, tiles, repeats=1):
    from contextlib import ExitStack

    import concourse.tile as tile
    from concourse import bacc, mybir

    f32 = mybir.dt.float32
    f32r = mybir.dt.float32r
    bf16 = mybir.dt.bfloat16
    i16 = mybir.dt.int16

    nc = bacc.Bacc("TRN2", target_bir_lowering=False, debug=False,
                   num_devices=NCORES)

    qt_d = nc.dram_tensor("qt", [HPC // 2, 128, S], bf16,
                          kind="ExternalInput").ap()
    kt_d = nc.dram_tensor("kt", [HPC // 2, 128, S], bf16,
                          kind="ExternalInput").ap()
    v_d = nc.dram_tensor("v", [HPC, 128, NKB * (D + 1)], bf16,
                         kind="ExternalInput").ap()
    out_d = nc.dram_tensor("out", [HPC, D + 1, S], f32,
                           kind="ExternalOutput").ap()
    data_idx = {}
    for ti, t in enumerate(tiles):
        if t[0] == 'data':
            data_idx[ti] = len(data_idx)
    if data_idx:
        mt_d = nc.dram_tensor("mt", [len(data_idx), KB, 2 * GSIZE * QT], f32,
                              kind="ExternalInput").ap()
    if _DBG:
        dbgb_d = nc.dram_tensor("dbgb", [KB, 2 * GSIZE * QT], f32,
                                kind="ExternalOutput").ap()
        dbgp_d = nc.dram_tensor("dbgp", [KB, 2 * GSIZE * QT], f32,
                                kind="ExternalOutput").ap()

    Exp = mybir.ActivationFunctionType.Exp
    MULT = mybir.AluOpType.mult
    ADD = mybir.AluOpType.add

    with tile.TileContext(nc) as tc, ExitStack() as ctx:
        qk_pool = ctx.enter_context(tc.tile_pool(name="qk", bufs=2))
        v_pool = ctx.enter_context(tc.tile_pool(name="vp", bufs=4))
        st_pool = ctx.enter_context(tc.tile_pool(name="st", bufs=3,
                                                 space="PSUM"))
        pt_pool = ctx.enter_context(tc.tile_pool(name="pt", bufs=6))
        acc_pool = ctx.enter_context(tc.tile_pool(name="acc", bufs=2,
                                                  space="PSUM"))
        out_pool = ctx.enter_context(tc.tile_pool(name="ob", bufs=4))
        b_pool = ctx.enter_context(tc.tile_pool(name="bp", bufs=1))

        # bias tiles: [KB, 2, w, QT] = (head, kblock-in-group, q) with
        # B16 where attention allowed, -1e30 where masked
        b_tiles = {}
        for ti, t in enumerate(tiles):
            if t[0] != 'causal':
                continue
            rel, w = t[1], t[2]
            m = b_pool.tile([KB, 2 * GSIZE * QT], f32, tag=f"b{ti}",
                            name=f"b{ti}")
            nc.gpsimd.memset(m[:], B16)
            m4 = m[:].rearrange("p (h g y) -> p h g y", h=2, y=QT)[:, :, :w, :]
            nc.gpsimd.affine_select(
                out=m4, in_=m4,
                compare_op=mybir.AluOpType.is_ge,
                fill=NEG, base=-rel,
                pattern=[[0, 2], [-KB, w], [1, QT]],
                channel_multiplier=-1)
            b_tiles[ti] = m
        m01 = b_pool.tile([KB, 2 * GSIZE * QT], bf16, tag="m01", name="m01")
        nc.gpsimd.memset(m01[:], 1.0)
        m01v = m01[:].rearrange("p (h g y) -> p h g y", h=2, y=QT)
        nc.gpsimd.affine_select(
            out=m01v, in_=m01v,
            compare_op=mybir.AluOpType.is_ge,
            fill=0.0, base=0,
            pattern=[[0, 2], [-KB, GSIZE], [1, QT]],
            channel_multiplier=-1)

        def _load_data_tiles():
            for ti, di in data_idx.items():
                m = b_pool.tile([KB, 2 * GSIZE * QT], f32, tag=f"b{ti}",
                                name=f"bd{ti}")
                nc.sync.dma_start(m[:], mt_d[di])
                b_tiles[ti] = m

        for _i in range(3):
            _t = st_pool.tile([128, 2 * GSIZE * QT], f32, tag="s",
                              name=f"sti{_i}")
            nc.vector.memset(_t[:], 0.0)

        for rep in range(repeats):
          for pair in range(HPC // 2):
            kt_c, qt_c = [], []
            v_ts = []
            for c in range(S // 512):
                kt1 = qk_pool.tile([128, 512], bf16, tag=f"kt{c}",
                                   name=f"kt{pair}_{c}")
                nc.sync.dma_start(kt1[:], kt_d[pair, :, c * 512:(c + 1) * 512])
                kt_c.append(kt1)
                qt1 = qk_pool.tile([128, 512], bf16, tag=f"qt{c}",
                                   name=f"qt{pair}_{c}")
                nc.sync.dma_start(qt1[:], qt_d[pair, :, c * 512:(c + 1) * 512])
                qt_c.append(qt1)
                if c == 0:
                    for sub in range(2):
                        h = 2 * pair + sub
                        v_t = v_pool.tile([128, NKB * (D + 1)], bf16, tag="v",
                                          name=f"v{h}")
                        nc.sync.dma_start(v_t[:], v_d[h])
                        v_ts.append(v_t)
                    if pair == 0 and rep == 0 and data_idx:
                        _load_data_tiles()

            # software pipeline: emit QK(g) one group ahead of PV(g-1)
            prev = None  # (groups_elems, acc, j)

            def emit_pv(pv):
                # start=True arms/resets the WHOLE PSUM bank, so only the
                # very first matmul into the acc bank may set it; head B's
                # first write then lands on the armed/zeroed region.
                st, pt, i0, w, first, last, acc, diag = pv
                for gp in range(w):
                    i = i0 + gp
                    y0 = gp * KB if (diag and not first) else 0
                    for sub in range(2):
                        nc.tensor.matmul(
                            acc[:, sub * QT + y0:(sub + 1) * QT],
                            lhsT=v_ts[sub][:, i * (D + 1):(i + 1) * (D + 1)],
                            rhs=pt[:, (sub * w + gp) * QT + y0:
                                   (sub * w + gp + 1) * QT],
                            start=(first and gp == 0 and sub == 0),
                            stop=(last and gp == w - 1 and sub == 1),
                            skip_group_check=True)

            for j in range(NQT):
                active = plan[j]
                acc = acc_pool.tile([D + 1, 2 * QT], f32, tag="a")
                jc, qo = j // 2, (j % 2) * 256
                for gi, (i0, w, kind, pat) in enumerate(active):
                    st = st_pool.tile([128, 2 * GSIZE * QT], f32, tag="s")
                    # causal diag: kblock gp has live q only from gp*KB
                    diag = (kind == 'mixed' and tiles[pat][0] == 'causal'
                            and tiles[pat][1] == 0)
                    # QK^T: A/B interleaved -> PE row tiles T0/T8 overlap
                    for gp in range(w):
                        i = i0 + gp
                        ci, ko = i // 4, (i % 4) * KB
                        y0 = gp * KB if diag else 0
                        for sub in range(2):
                            po = 64 * sub
                            nc.tensor.matmul(
                                st[:, (sub * w + gp) * QT + y0:
                                   (sub * w + gp + 1) * QT],
                                lhsT=kt_c[ci][po:po + 64, ko:ko + KB],
                                rhs=qt_c[jc][po:po + 64, qo + y0:qo + QT],
                                start=True, stop=True)
                    # PV for the previous group (keeps PE fed while the
                    # exp for this group runs)
                    if prev is not None:
                        emit_pv(prev)
                        prev = None
                    # exp (both heads in one instruction)
                    pt = pt_pool.tile([128, 2 * GSIZE * QT], bf16, tag="p")
                    n = 2 * w * QT
                    if kind == 'mixed' and diag and j <= 1:
                        # error-critical early rows: exact exp + 0/1 mask
                        nc.scalar.activation(pt[:, :n], st[:, :n], Exp,
                                             scale=SCALE)
                        nc.vector.tensor_mul(pt[:, :n], pt[:, :n],
                                             m01[:, :n])
                    elif kind == 'mixed':
                        nc.vector.scalar_tensor_tensor(
                            pt[:, :n].bitcast(i16), st[:, :n], A16,
                            b_tiles[pat][:, :n], MULT, ADD)
                    elif (j, gi) in DVE_FULL:
                        nc.vector.tensor_scalar(
                            pt[:, :n].bitcast(i16), st[:, :n], A16, B16,
                            MULT, ADD)
                    else:
                        nc.scalar.activation(pt[:, :n], st[:, :n], Exp,
                                             scale=SCALE)
                    if _DBG and rep == 0 and pair == 0 and j == 0 and gi == 0:
                        nc.sync.dma_start(dbgb_d, b_tiles[pat][:])
                        dbgf = out_pool.tile([KB, 2 * GSIZE * QT], f32,
                                             tag="dbgf", name="dbgf")
                        nc.vector.tensor_copy(dbgf[:], pt[:])
                        nc.sync.dma_start(dbgp_d, dbgf[:])
                    prev = (st, pt, i0, w, gi == 0, gi == len(active) - 1,
                            acc, diag)
                    if _NO_PIPE:
                        emit_pv(prev)
                        prev = None
                if prev is not None:
                    emit_pv(prev)
                    prev = None
                # evacuate PSUM -> SBUF (alternating engine), DMA out
                ob = out_pool.tile([D + 1, 2 * QT], f32, tag="o")
                if active:
                    if j % 2 == 0:
                        nc.scalar.copy(ob[:], acc[:])
                    else:
                        nc.vector.tensor_copy(ob[:], acc[:])
                else:
                    nc.vector.memset(ob[:], 0.0)
                for sub in range(2):
                    nc.sync.dma_start(
                        out_d[2 * pair + sub, :, j * QT:(j + 1) * QT],
                        ob[:, sub * QT:(sub + 1) * QT])

    nc.compile()
    return nc


def _get_nc(mask):
    key = mask.tobytes()
    if key not in _CACHE:
        plan, tiles = _plan_from_mask(mask)
        nc = _build(plan, tiles)
        _CACHE[key] = (nc, tiles)
    return _CACHE[key]


def kernel(q, k, v, mask, _trace=False):
    import jax.numpy as jnp
    from concourse.bass_utils import run_bass_kernel_spmd

    mask = np.asarray(mask).astype(bool)
    q = np.asarray(q, dtype=np.float32).reshape(B * H, S, D)
    k = np.asarray(k, dtype=np.float32).reshape(B * H, S, D)
    v = np.asarray(v, dtype=np.float32).reshape(B * H, S, D)

    nc, tiles = _get_nc(mask)
    mt = _stack_bias_tiles(tiles)

    in_maps = []
    for c in range(NCORES):
        sl = slice(HPC * c, HPC * (c + 1))
        qc = np.asarray(jnp.asarray(np.ascontiguousarray(
            q[sl].transpose(0, 2, 1)).reshape(HPC // 2, 128, S),
            dtype=jnp.bfloat16))
        kc = np.asarray(jnp.asarray(np.ascontiguousarray(
            k[sl].transpose(0, 2, 1)).reshape(HPC // 2, 128, S),
            dtype=jnp.bfloat16))
        vc = np.concatenate(
            [v[sl], np.ones((HPC, S, 1), dtype=np.float32)], axis=2)
        vc = vc.reshape(HPC, NKB, KB, D + 1).transpose(0, 2, 1, 3)
        vc = np.ascontiguousarray(vc).reshape(HPC, KB, NKB * (D + 1))
        vc = np.asarray(jnp.asarray(vc, dtype=jnp.bfloat16))
        m = {"qt": qc, "kt": kc, "v": vc}
        if mt is not None:
            m["mt"] = mt
        in_maps.append(m)

    res = run_bass_kernel_spmd(nc, in_maps, core_ids=list(range(NCORES)),
                               trace=_trace)

    outs = []
    for c in range(NCORES):
        o = res.results[c]["out"]  # [HPC, D+1, S]
        num = o[:, :D, :]
        den = o[:, D:D + 1, :]
        with np.errstate(invalid='ignore', divide='ignore'):
            outs.append((num / den).transpose(0, 2, 1))  # [HPC, S, D]
    full = np.concatenate(outs, axis=0).reshape(B, H, S, D).astype(np.float32)
    if _trace:
        return full, res
    return full
